# revision 5
# baseline (speedup 1.0000x reference)
"""Trainium2 Bass kernel: ConvNeXt MLP + parallel MoE-LoRA (data-parallel over tokens).

Math per token t (D=512, Dh=2048, E=3 experts, r=8, top-k=2):
    base = gelu(x @ W1 + b1) @ W2 + b2
    g_e  = gelu(x @ w_down[e]) * wts[e, t]          (wts from top-k routing)
    out  = base + sum_e g_e @ w_up[e]

Strategy (per NeuronCore, 8 cores data-parallel on the token dim):
  - tokens tiled 128 at a time; supergroups of 4 tiles (512 tokens) so the
    MM1 moving free dim is 512.
  - x tile [128t, 512d] is PE-transposed into xT [128d, t] chunks.
  - MM1: hT[h,t] = W1_chunk.T @ xT   (feature-major hidden), fused
    bias+gelu on ScalarE into actT.
  - MM2: out[t,d] accumulates b2 (rank-1 ones matmul) + 16 h-chunks
    (lhsT = actT slices) + the MoE-LoRA rank-24 matmul, all in one PSUM
    accumulation group.
  - LoRA: g[t,24] = gelu(xT.T @ wdown_all), scaled per-expert by routing
    weights (per-partition scalars), PE-transposed, matmul'd with
    wup_all[24,512] into the same PSUM accumulator.
  - routing weights wts[e,t] = sum_k probs[t,k]*(idx[t,k]==e) computed on
    device in a small DVE prologue over all tokens at once.
  - big matmuls use dtype float32r (full PE rate at N>=256 on trn2);
    fp32r operands are produced by rounding compute ops (DVE/ACT writes),
    as the BIR verifier requires.
"""

import os
import numpy as np

P = 128
D = 512
DH = 2048
E = 3
R = 8
ER = E * R  # 24
NH = DH // P  # 16
NDC = D // P  # 4
N_CORES = 8
T_FULL = 64 * 28 * 28  # 50176
TC = T_FULL // N_CORES  # 6272
GROUP_TILES = 4

_CACHE = {}


def _build(tc_tokens, use_gelu=True):
    import concourse.bacc as bacc
    import concourse.tile as tile
    import concourse.mybir as mybir
    from contextlib import ExitStack

    f32 = mybir.dt.float32
    f32r = mybir.dt.float32r
    i32 = mybir.dt.int32
    AF = mybir.ActivationFunctionType
    act_fn = AF.Gelu if use_gelu else AF.Relu
    OP = mybir.AluOpType

    nt = tc_tokens // P  # token tiles
    assert tc_tokens % P == 0

    nc = bacc.Bacc("TRN2", target_bir_lowering=False, debug=False,
                   num_devices=N_CORES)

    x = nc.dram_tensor("x", [tc_tokens, D], f32, kind="ExternalInput").ap()
    w1 = nc.dram_tensor("w1", [D, DH], f32, kind="ExternalInput").ap()
    w2 = nc.dram_tensor("w2", [DH, D], f32, kind="ExternalInput").ap()
    b1 = nc.dram_tensor("b1", [DH], f32, kind="ExternalInput").ap()
    b2 = nc.dram_tensor("b2", [D], f32, kind="ExternalInput").ap()
    wd = nc.dram_tensor("wd", [D, ER], f32, kind="ExternalInput").ap()
    wu = nc.dram_tensor("wu", [ER, D], f32, kind="ExternalInput").ap()
    tkp = nc.dram_tensor("tkp", [tc_tokens, 2], f32, kind="ExternalInput").ap()
    tki = nc.dram_tensor("tki", [tc_tokens, 4], i32, kind="ExternalInput").ap()
    ident_d = nc.dram_tensor("ident", [P, P], f32, kind="ExternalInput").ap()
    out = nc.dram_tensor("out", [tc_tokens, D], f32, kind="ExternalOutput").ap()

    with tile.TileContext(nc) as tc, ExitStack() as ctx:
        cons = ctx.enter_context(tc.tile_pool(name="cons", bufs=1))
        stg = ctx.enter_context(tc.tile_pool(name="stg", bufs=2))
        xin = ctx.enter_context(tc.tile_pool(name="xin", bufs=6))
        xtp = ctx.enter_context(tc.tile_pool(name="xtp", bufs=2))
        actp = ctx.enter_context(tc.tile_pool(name="actp", bufs=2))
        outp = ctx.enter_context(tc.tile_pool(name="outp", bufs=4))
        gp = ctx.enter_context(tc.tile_pool(name="gp", bufs=2))
        ps_xt = ctx.enter_context(tc.tile_pool(name="ps_xt", bufs=2, space="PSUM"))
        ps_h = ctx.enter_context(tc.tile_pool(name="ps_h", bufs=2, space="PSUM"))
        ps_o = ctx.enter_context(tc.tile_pool(name="ps_o", bufs=2, space="PSUM"))
        ps_g = ctx.enter_context(tc.tile_pool(name="ps_g", bufs=1, space="PSUM"))
        ps_gt = ctx.enter_context(tc.tile_pool(name="ps_gt", bufs=1, space="PSUM"))

        # ---- constants / weights (staged fp32 DMA -> fp32r rounding copy) ----
        ident_sb = cons.tile([P, P], f32)
        nc.sync.dma_start(ident_sb[:], ident_d)
        ident_r = cons.tile([P, P], f32r)
        nc.vector.tensor_copy(ident_r[:], ident_sb[:])

        W1r = cons.tile([P, NDC * DH], f32r)
        for c in range(NDC):
            s = stg.tile([P, DH], f32, tag="stage", name=f"stg_w1_{c}")
            nc.sync.dma_start(s[:], w1[c * P:(c + 1) * P, :])
            nc.vector.tensor_copy(W1r[:, c * DH:(c + 1) * DH], s[:])
        W2r = cons.tile([P, NH * D], f32r)
        for h in range(NH):
            s = stg.tile([P, D], f32, tag="stage", name=f"stg_w2_{h}")
            nc.sync.dma_start(s[:], w2[h * P:(h + 1) * P, :])
            nc.vector.tensor_copy(W2r[:, h * D:(h + 1) * D], s[:])
        wdr = cons.tile([P, NDC * ER], f32r)
        for c in range(NDC):
            s = stg.tile([P, ER], f32, tag="stage", name=f"stg_wd_{c}")
            nc.sync.dma_start(s[:], wd[c * P:(c + 1) * P, :])
            nc.vector.tensor_copy(wdr[:, c * ER:(c + 1) * ER], s[:])
        wur = cons.tile([ER, D], f32r)
        s = stg.tile([ER, D], f32, tag="stage", name="stg_wu")
        nc.sync.dma_start(s[:], wu)
        nc.vector.tensor_copy(wur[:], s[:])
        b2r = cons.tile([1, D], f32r)
        s = stg.tile([1, D], f32, tag="stage", name="stg_b2")
        nc.sync.dma_start(s[:], b2.rearrange("(o d) -> o d", o=1))
        nc.vector.tensor_copy(b2r[:], s[:])
        b1_sb = cons.tile([P, NH], f32)
        nc.sync.dma_start(b1_sb[:], b1.rearrange("(c p) -> p c", p=P))
        ones_sb = cons.tile([1, P], f32)
        nc.vector.memset(ones_sb[:], 1.0)
        ones_r = cons.tile([1, P], f32r)
        nc.vector.tensor_copy(ones_r[:], ones_sb[:])

        # ---- routing weights prologue: wts[e, tile, p] ----
        tp_sb = cons.tile([P, nt * 2], f32)
        nc.sync.dma_start(tp_sb.rearrange("p (n k) -> p n k", k=2),
                          tkp.rearrange("(n p) k -> p n k", p=P))
        ti_sb = cons.tile([P, nt * 4], i32)
        nc.sync.dma_start(ti_sb.rearrange("p (n k) -> p n k", k=4),
                          tki.rearrange("(n p) k -> p n k", p=P))
        idxf = cons.tile([P, nt * 2], f32)
        iv = ti_sb.rearrange("p (n k two) -> p n k two", k=2, two=2)
        nc.vector.tensor_copy(
            idxf.rearrange("p (n k one) -> p n k one", k=2, one=1),
            iv[:, :, :, 0:1])
        wts = cons.tile([P, E * nt], f32)
        for e in range(E):
            eq = cons.tile([P, nt * 2], f32, tag="eq", name=f"eq{e}")
            nc.vector.tensor_scalar(eq[:], idxf[:], float(e), None,
                                    op0=OP.is_equal)
            nc.vector.tensor_tensor(eq[:], eq[:], tp_sb[:], op=OP.mult)
            nc.vector.tensor_reduce(wts[:, e * nt:(e + 1) * nt],
                                    eq.rearrange("p (n k) -> p n k", k=2),
                                    axis=mybir.AxisListType.X, op=OP.add)

        # ---- main loop over supergroups ----
        t0 = 0
        while t0 < nt:
            ng = min(GROUP_TILES, nt - t0)
            G = ng * P

            x_ts = []
            for i in range(ng):
                x_t = xin.tile([P, D], f32, tag="x_t", name=f"x_t{t0 + i}")
                nc.sync.dma_start(x_t[:], x[(t0 + i) * P:(t0 + i + 1) * P, :])
                x_ts.append(x_t)

            # transpose x -> xT [128d, (c, t)], rounded to fp32r by the copy
            xT = xtp.tile([P, NDC * G], f32r, tag="xT", name=f"xT{t0}")
            for i in range(ng):
                px = ps_xt.tile([P, 512], f32, tag="ps_x", name=f"ps_x{t0 + i}")
                for c in range(NDC):
                    nc.tensor.matmul(px[:, c * P:(c + 1) * P],
                                     x_ts[i][:, c * P:(c + 1) * P],
                                     ident_sb[:],
                                     is_transpose=True,
                                     start=(c == 0), stop=(c == NDC - 1))
                nc.vector.tensor_copy(
                    xT.rearrange("p (c g) -> p c g", c=NDC)[:, :, i * P:(i + 1) * P],
                    px.rearrange("p (c g) -> p c g", c=NDC))

            # MM1 + bias + gelu -> actT [128h, (h_chunk, t)]
            actT = actp.tile([P, NH * G], f32r, tag="actT", name=f"actT{t0}")
            for h in range(NH):
                ph = ps_h.tile([P, 512], f32, tag="ps_hT", name=f"ps_hT{t0}_{h}")
                for c in range(NDC):
                    nc.tensor.matmul(
                        ph[:, :G],
                        W1r[:, c * DH + h * P: c * DH + (h + 1) * P],
                        xT[:, c * G:(c + 1) * G],
                        start=(c == 0), stop=(c == NDC - 1))
                nc.scalar.activation(actT[:, h * G:(h + 1) * G], ph[:, :G],
                                     act_fn, bias=b1_sb[:, h:h + 1], scale=1.0)

            # MM2 + LoRA per token tile
            for i in range(ng):
                tt = t0 + i
                po = ps_o.tile([P, D], f32, tag="ps_out", name=f"ps_out{tt}")
                # bias b2 via rank-1 matmul (starts the accumulation group)
                nc.tensor.matmul(po[:], ones_r[:], b2r[:],
                                 start=True, stop=False)
                for h in range(NH):
                    nc.tensor.matmul(
                        po[:],
                        actT[:, h * G + i * P: h * G + (i + 1) * P],
                        W2r[:, h * D:(h + 1) * D],
                        start=False, stop=False)
                # LoRA down: g[t, 24]
                pg = ps_g.tile([P, ER], f32, tag="ps_lg", name=f"ps_lg{tt}")
                for c in range(NDC):
                    nc.tensor.matmul(
                        pg[:],
                        xT[:, c * G + i * P: c * G + (i + 1) * P],
                        wdr[:, c * ER:(c + 1) * ER],
                        start=(c == 0), stop=(c == NDC - 1))
                g_sb = gp.tile([P, ER], f32, tag="g_sb", name=f"g_sb{tt}")
                nc.scalar.activation(g_sb[:], pg[:], act_fn)
                g2 = gp.tile([P, ER], f32r, tag="g2", name=f"g2{tt}")
                for e in range(E):
                    nc.vector.tensor_scalar(
                        g2[:, e * R:(e + 1) * R], g_sb[:, e * R:(e + 1) * R],
                        wts[:, e * nt + tt: e * nt + tt + 1], None, op0=OP.mult)
                pgt = ps_gt.tile([ER, P], f32r, tag="ps_lgt", name=f"ps_lgt{tt}")
                nc.tensor.matmul(pgt[:], g2[:], ident_r[:], is_transpose=True)
                gt_sb = gp.tile([ER, P], f32r, tag="gt_sb", name=f"gt_sb{tt}")
                nc.vector.tensor_copy(gt_sb[:], pgt[:])
                # LoRA up into the same accumulator (closes the group)
                nc.tensor.matmul(po[:], gt_sb[:], wur[:],
                                 start=False, stop=True)

                o_sb = outp.tile([P, D], f32, tag="o_sb", name=f"o_sb{tt}")
                nc.vector.tensor_copy(o_sb[:], po[:])
                nc.sync.dma_start(out[tt * P:(tt + 1) * P, :], o_sb[:])

            t0 += ng

    nc.compile()
    return nc


def _get_nc():
    key = ("full", TC)
    if key not in _CACHE:
        _CACHE[key] = _build(TC, use_gelu=True)
    return _CACHE[key]


def _make_in_maps(inputs, tc_tokens=TC, n_cores=N_CORES):
    x = np.ascontiguousarray(inputs["x"], dtype=np.float32)
    T = x.size // D
    x_flat = x.reshape(T, D)
    W1 = np.ascontiguousarray(inputs["W1"], dtype=np.float32)
    W2 = np.ascontiguousarray(inputs["W2"], dtype=np.float32)
    b1 = np.ascontiguousarray(inputs["b1"], dtype=np.float32)
    b2 = np.ascontiguousarray(inputs["b2"], dtype=np.float32)
    wdn = np.ascontiguousarray(
        np.asarray(inputs["w_down"], dtype=np.float32).transpose(1, 0, 2).reshape(D, ER))
    wup = np.ascontiguousarray(
        np.asarray(inputs["w_up"], dtype=np.float32).reshape(ER, D))
    tkp = np.ascontiguousarray(inputs["topk_probs"], dtype=np.float32)
    tki_in = np.asarray(inputs["topk_indices"])
    tki = np.zeros((T, 4), dtype=np.int32)
    tki[:, 0] = tki_in[:, 0]
    tki[:, 2] = tki_in[:, 1]
    ident = np.eye(P, dtype=np.float32)

    in_maps = []
    for c in range(n_cores):
        sl = slice(c * tc_tokens, (c + 1) * tc_tokens)
        in_maps.append(dict(
            x=np.ascontiguousarray(x_flat[sl]), w1=W1, w2=W2, b1=b1, b2=b2,
            wd=wdn, wu=wup, tkp=np.ascontiguousarray(tkp[sl]),
            tki=np.ascontiguousarray(tki[sl]), ident=ident))
    return in_maps


def _ensure_ntff_hook():
    """Register the axon NTFF profile hook if the image's antenv lacks it."""
    import sys
    import types
    try:
        from antenv.axon_hooks import get_axon_ntff_profile_hook  # noqa: F401
        return True
    except ImportError:
        pass
    try:
        from trn_agent_boot.trn_boot import _ntff_profile_via_ctypes
        mod = types.ModuleType("antenv.axon_hooks")
        _hook = [None]
        mod.set_axon_ntff_profile_hook = lambda h: _hook.__setitem__(0, h)
        mod.get_axon_ntff_profile_hook = lambda: _hook[0]
        sys.modules["antenv.axon_hooks"] = mod
        import antenv
        antenv.axon_hooks = mod
        mod.set_axon_ntff_profile_hook(
            _ntff_profile_via_ctypes("/opt/axon/libaxon_pjrt.so"))
        return True
    except Exception:
        return False


def kernel(**inputs):
    from concourse.bass_utils import run_bass_kernel_spmd

    nc = _get_nc()
    in_maps = _make_in_maps(inputs)
    trace = bool(int(os.environ.get("KERNEL_TRACE", "0")))
    if trace and not _ensure_ntff_hook():
        trace = False
    res = run_bass_kernel_spmd(nc, in_maps, list(range(N_CORES)), trace=trace)
    if trace:
        _CACHE["last_result"] = res
    out = np.concatenate([res.results[i]["out"] for i in range(N_CORES)], axis=0)
    return out.reshape(np.asarray(inputs["x"]).shape).astype(np.float32)


# revision 6
# speedup vs baseline: 1.2733x; 1.2733x over previous
"""Trainium2 Bass kernel: ConvNeXt MLP + parallel MoE-LoRA (data-parallel over tokens).

Math per token t (D=512, Dh=2048, E=3 experts, r=8, top-k=2):
    base = gelu(x @ W1 + b1) @ W2 + b2
    g_e  = gelu(x @ w_down[e]) * wts[e, t]          (wts from top-k routing)
    out  = base + sum_e g_e @ w_up[e]

Strategy (per NeuronCore, 8 cores data-parallel on the token dim):
  - tokens tiled 128 at a time; supergroups of 4 tiles (512 tokens) so the
    MM1 moving free dim is 512.
  - x tile [128t, 512d] is cast to bf16 and PE-transposed into xT [128d, t].
  - MM1: hT[h,t] = W1_chunk.T @ xT   (feature-major hidden), fused
    bias+gelu on ScalarE into actT (bf16).
  - MM2: out[t,d] accumulates 16 h-chunks (lhsT = actT slices) + the
    MoE-LoRA rank-24 matmul in one PSUM accumulation group; b2 is added
    during the PSUM->SBUF drain from a replicated [128,512] bias tile.
  - LoRA: g[t,24] = gelu(xT.T @ wdown_all), scaled per-expert by routing
    weights (per-partition scalars), PE-transposed, matmul'd with
    wup_all[24,512] into the same PSUM accumulator.
  - routing weights wts[e,t] = sum_k probs[t,k]*(idx[t,k]==e) computed on
    device in a small DVE prologue over all tokens at once.
  - matmuls run in bf16 (full PE rate, fast weight load); accumulation is
    always fp32 in PSUM.
"""

import os
import numpy as np

P = 128
D = 512
DH = 2048
E = 3
R = 8
ER = E * R  # 24
NH = DH // P  # 16
NDC = D // P  # 4
N_CORES = 8
T_FULL = 64 * 28 * 28  # 50176
TC = T_FULL // N_CORES  # 6272
GROUP_TILES = 4

_CACHE = {}


def _build(tc_tokens, use_gelu=True):
    import concourse.bacc as bacc
    import concourse.tile as tile
    import concourse.mybir as mybir
    from contextlib import ExitStack

    f32 = mybir.dt.float32
    bf16 = mybir.dt.bfloat16
    i32 = mybir.dt.int32
    AF = mybir.ActivationFunctionType
    act_fn = AF.Gelu if use_gelu else AF.Relu
    OP = mybir.AluOpType

    nt = tc_tokens // P  # token tiles
    assert tc_tokens % P == 0

    nc = bacc.Bacc("TRN2", target_bir_lowering=False, debug=False,
                   num_devices=N_CORES)

    x = nc.dram_tensor("x", [tc_tokens, D], f32, kind="ExternalInput").ap()
    w1 = nc.dram_tensor("w1", [D, DH], f32, kind="ExternalInput").ap()
    w2 = nc.dram_tensor("w2", [DH, D], f32, kind="ExternalInput").ap()
    b1 = nc.dram_tensor("b1", [DH], f32, kind="ExternalInput").ap()
    b2 = nc.dram_tensor("b2", [D], f32, kind="ExternalInput").ap()
    wd = nc.dram_tensor("wd", [D, ER], f32, kind="ExternalInput").ap()
    wu = nc.dram_tensor("wu", [ER, D], f32, kind="ExternalInput").ap()
    tkp = nc.dram_tensor("tkp", [tc_tokens, 2], f32, kind="ExternalInput").ap()
    tki = nc.dram_tensor("tki", [tc_tokens, 4], i32, kind="ExternalInput").ap()
    ident_d = nc.dram_tensor("ident", [P, P], f32, kind="ExternalInput").ap()
    out = nc.dram_tensor("out", [tc_tokens, D], f32, kind="ExternalOutput").ap()

    with tile.TileContext(nc) as tc, ExitStack() as ctx:
        cons = ctx.enter_context(tc.tile_pool(name="cons", bufs=1))
        stg = ctx.enter_context(tc.tile_pool(name="stg", bufs=2))
        xin = ctx.enter_context(tc.tile_pool(name="xin", bufs=6))
        xbp = ctx.enter_context(tc.tile_pool(name="xbp", bufs=4))
        xtp = ctx.enter_context(tc.tile_pool(name="xtp", bufs=2))
        actp = ctx.enter_context(tc.tile_pool(name="actp", bufs=2))
        outp = ctx.enter_context(tc.tile_pool(name="outp", bufs=4))
        gp = ctx.enter_context(tc.tile_pool(name="gp", bufs=2))
        ps_xt = ctx.enter_context(tc.tile_pool(name="ps_xt", bufs=2, space="PSUM"))
        ps_h = ctx.enter_context(tc.tile_pool(name="ps_h", bufs=2, space="PSUM"))
        ps_o = ctx.enter_context(tc.tile_pool(name="ps_o", bufs=2, space="PSUM"))
        ps_g = ctx.enter_context(tc.tile_pool(name="ps_g", bufs=1, space="PSUM"))
        ps_gt = ctx.enter_context(tc.tile_pool(name="ps_gt", bufs=1, space="PSUM"))

        # ---- identity (needed by the very first transposes) ----
        ident_sb = cons.tile([P, P], f32)
        nc.sync.dma_start(ident_sb[:], ident_d)
        ident_b = cons.tile([P, P], bf16)
        nc.vector.tensor_copy(ident_b[:], ident_sb[:])

        # ---- prefetch x tiles of group 0 before the weight DMAs ----
        prefetched = {}
        for i in range(min(GROUP_TILES, nt)):
            x_t = xin.tile([P, D], f32, tag="x_t", name=f"x_t{i}")
            nc.sync.dma_start(x_t[:], x[i * P:(i + 1) * P, :])
            prefetched[i] = x_t

        # ---- weights: staged fp32 DMA -> bf16 cast ----
        W1s = cons.tile([P, NDC * DH], bf16)
        for c in range(NDC):
            s = stg.tile([P, DH], f32, tag="stage", name=f"stg_w1_{c}")
            nc.sync.dma_start(s[:], w1[c * P:(c + 1) * P, :])
            nc.vector.tensor_copy(W1s[:, c * DH:(c + 1) * DH], s[:])
        W2s = cons.tile([P, NH * D], bf16)
        for h in range(NH):
            s = stg.tile([P, D], f32, tag="stage", name=f"stg_w2_{h}")
            nc.sync.dma_start(s[:], w2[h * P:(h + 1) * P, :])
            nc.vector.tensor_copy(W2s[:, h * D:(h + 1) * D], s[:])
        wdr = cons.tile([P, NDC * ER], bf16)
        for c in range(NDC):
            s = stg.tile([P, ER], f32, tag="stage", name=f"stg_wd_{c}")
            nc.sync.dma_start(s[:], wd[c * P:(c + 1) * P, :])
            nc.vector.tensor_copy(wdr[:, c * ER:(c + 1) * ER], s[:])
        wur = cons.tile([ER, D], bf16)
        s = stg.tile([ER, D], f32, tag="stage", name="stg_wu")
        nc.sync.dma_start(s[:], wu)
        nc.vector.tensor_copy(wur[:], s[:])
        b1_sb = cons.tile([P, NH], f32)
        nc.sync.dma_start(b1_sb[:], b1.rearrange("(c p) -> p c", p=P))

        # ---- replicated b2 [128, 512] via rank-1 ones x b2 matmul ----
        ones_b = cons.tile([1, P], bf16)
        nc.vector.memset(ones_b[:], 1.0)
        b2s = stg.tile([1, D], f32, tag="stage", name="stg_b2")
        nc.sync.dma_start(b2s[:], b2.rearrange("(o d) -> o d", o=1))
        b2b = cons.tile([1, D], bf16)
        nc.vector.tensor_copy(b2b[:], b2s[:])
        ps_b2 = ps_o.tile([P, D], f32, tag="ps_out", name="ps_b2rep")
        nc.tensor.matmul(ps_b2[:], ones_b[:], b2b[:], start=True, stop=True)
        b2rep = cons.tile([P, D], f32)
        nc.vector.tensor_copy(b2rep[:], ps_b2[:])

        # ---- routing weights prologue: wts[e, tile, p] ----
        tp_sb = cons.tile([P, nt * 2], f32)
        nc.sync.dma_start(tp_sb.rearrange("p (n k) -> p n k", k=2),
                          tkp.rearrange("(n p) k -> p n k", p=P))
        ti_sb = cons.tile([P, nt * 4], i32)
        nc.sync.dma_start(ti_sb.rearrange("p (n k) -> p n k", k=4),
                          tki.rearrange("(n p) k -> p n k", p=P))
        idxf = cons.tile([P, nt * 2], f32)
        iv = ti_sb.rearrange("p (n k two) -> p n k two", k=2, two=2)
        nc.vector.tensor_copy(
            idxf.rearrange("p (n k one) -> p n k one", k=2, one=1),
            iv[:, :, :, 0:1])
        wts = cons.tile([P, E * nt], f32)
        for e in range(E):
            eq = cons.tile([P, nt * 2], f32, tag="eq", name=f"eq{e}")
            nc.vector.tensor_scalar(eq[:], idxf[:], float(e), None,
                                    op0=OP.is_equal)
            nc.vector.tensor_tensor(eq[:], eq[:], tp_sb[:], op=OP.mult)
            nc.vector.tensor_reduce(wts[:, e * nt:(e + 1) * nt],
                                    eq.rearrange("p (n k) -> p n k", k=2),
                                    axis=mybir.AxisListType.X, op=OP.add)

        # ---- main loop over supergroups ----
        t0 = 0
        while t0 < nt:
            ng = min(GROUP_TILES, nt - t0)
            G = ng * P

            x_ts = []
            for i in range(ng):
                tt = t0 + i
                if tt in prefetched:
                    x_t = prefetched.pop(tt)
                else:
                    x_t = xin.tile([P, D], f32, tag="x_t", name=f"x_t{tt}")
                    nc.sync.dma_start(x_t[:], x[tt * P:(tt + 1) * P, :])
                x_ts.append(x_t)

            # cast x to bf16, transpose -> xT [128d, (c, t)] bf16
            xT = xtp.tile([P, NDC * G], bf16, tag="xT", name=f"xT{t0}")
            for i in range(ng):
                xb = xbp.tile([P, D], bf16, tag="xb", name=f"xb{t0 + i}")
                nc.vector.tensor_copy(xb[:], x_ts[i][:])
                px = ps_xt.tile([P, 512], bf16, tag="ps_x", name=f"ps_x{t0 + i}")
                for c in range(NDC):
                    nc.tensor.matmul(px[:, c * P:(c + 1) * P],
                                     xb[:, c * P:(c + 1) * P],
                                     ident_b[:],
                                     is_transpose=True,
                                     start=(c == 0), stop=(c == NDC - 1))
                nc.vector.tensor_copy(
                    xT.rearrange("p (c g) -> p c g", c=NDC)[:, :, i * P:(i + 1) * P],
                    px.rearrange("p (c g) -> p c g", c=NDC))

            # MM1 + bias + gelu -> actT [128h, (h_chunk, t)] bf16
            actT = actp.tile([P, NH * G], bf16, tag="actT", name=f"actT{t0}")
            for h in range(NH):
                ph = ps_h.tile([P, 512], f32, tag="ps_hT", name=f"ps_hT{t0}_{h}")
                for c in range(NDC):
                    nc.tensor.matmul(
                        ph[:, :G],
                        W1s[:, c * DH + h * P: c * DH + (h + 1) * P],
                        xT[:, c * G:(c + 1) * G],
                        start=(c == 0), stop=(c == NDC - 1))
                nc.scalar.activation(actT[:, h * G:(h + 1) * G], ph[:, :G],
                                     act_fn, bias=b1_sb[:, h:h + 1], scale=1.0)

            # MM2 + LoRA per token tile
            for i in range(ng):
                tt = t0 + i
                po = ps_o.tile([P, D], f32, tag="ps_out", name=f"ps_out{tt}")
                for h in range(NH):
                    nc.tensor.matmul(
                        po[:],
                        actT[:, h * G + i * P: h * G + (i + 1) * P],
                        W2s[:, h * D:(h + 1) * D],
                        start=(h == 0), stop=False)
                # LoRA down: g[t, 24]
                pg = ps_g.tile([P, ER], f32, tag="ps_lg", name=f"ps_lg{tt}")
                for c in range(NDC):
                    nc.tensor.matmul(
                        pg[:],
                        xT[:, c * G + i * P: c * G + (i + 1) * P],
                        wdr[:, c * ER:(c + 1) * ER],
                        start=(c == 0), stop=(c == NDC - 1))
                g_sb = gp.tile([P, ER], f32, tag="g_sb", name=f"g_sb{tt}")
                nc.scalar.activation(g_sb[:], pg[:], act_fn)
                g2 = gp.tile([P, ER], bf16, tag="g2", name=f"g2{tt}")
                for e in range(E):
                    nc.vector.tensor_scalar(
                        g2[:, e * R:(e + 1) * R], g_sb[:, e * R:(e + 1) * R],
                        wts[:, e * nt + tt: e * nt + tt + 1], None, op0=OP.mult)
                pgt = ps_gt.tile([ER, P], bf16, tag="ps_lgt", name=f"ps_lgt{tt}")
                nc.tensor.matmul(pgt[:], g2[:], ident_b[:], is_transpose=True)
                gt_sb = gp.tile([ER, P], bf16, tag="gt_sb", name=f"gt_sb{tt}")
                nc.vector.tensor_copy(gt_sb[:], pgt[:])
                # LoRA up into the same accumulator (closes the group)
                nc.tensor.matmul(po[:], gt_sb[:], wur[:],
                                 start=False, stop=True)

                o_sb = outp.tile([P, D], f32, tag="o_sb", name=f"o_sb{tt}")
                nc.vector.tensor_tensor(o_sb[:], po[:], b2rep[:], op=OP.add)
                nc.sync.dma_start(out[tt * P:(tt + 1) * P, :], o_sb[:])

            t0 += ng

    nc.compile()
    return nc


def _get_nc():
    key = ("full", TC)
    if key not in _CACHE:
        _CACHE[key] = _build(TC, use_gelu=True)
    return _CACHE[key]


def _make_in_maps(inputs, tc_tokens=TC, n_cores=N_CORES):
    x = np.ascontiguousarray(inputs["x"], dtype=np.float32)
    T = x.size // D
    x_flat = x.reshape(T, D)
    W1 = np.ascontiguousarray(inputs["W1"], dtype=np.float32)
    W2 = np.ascontiguousarray(inputs["W2"], dtype=np.float32)
    b1 = np.ascontiguousarray(inputs["b1"], dtype=np.float32)
    b2 = np.ascontiguousarray(inputs["b2"], dtype=np.float32)
    wdn = np.ascontiguousarray(
        np.asarray(inputs["w_down"], dtype=np.float32).transpose(1, 0, 2).reshape(D, ER))
    wup = np.ascontiguousarray(
        np.asarray(inputs["w_up"], dtype=np.float32).reshape(ER, D))
    tkp = np.ascontiguousarray(inputs["topk_probs"], dtype=np.float32)
    tki_in = np.asarray(inputs["topk_indices"])
    tki = np.zeros((T, 4), dtype=np.int32)
    tki[:, 0] = tki_in[:, 0]
    tki[:, 2] = tki_in[:, 1]
    ident = np.eye(P, dtype=np.float32)

    in_maps = []
    for c in range(n_cores):
        sl = slice(c * tc_tokens, (c + 1) * tc_tokens)
        in_maps.append(dict(
            x=np.ascontiguousarray(x_flat[sl]), w1=W1, w2=W2, b1=b1, b2=b2,
            wd=wdn, wu=wup, tkp=np.ascontiguousarray(tkp[sl]),
            tki=np.ascontiguousarray(tki[sl]), ident=ident))
    return in_maps


def _ensure_ntff_hook():
    """Register the axon NTFF profile hook if the image's antenv lacks it."""
    import sys
    import types
    try:
        from antenv.axon_hooks import get_axon_ntff_profile_hook  # noqa: F401
        return True
    except ImportError:
        pass
    try:
        from trn_agent_boot.trn_boot import _ntff_profile_via_ctypes
        mod = types.ModuleType("antenv.axon_hooks")
        _hook = [None]
        mod.set_axon_ntff_profile_hook = lambda h: _hook.__setitem__(0, h)
        mod.get_axon_ntff_profile_hook = lambda: _hook[0]
        sys.modules["antenv.axon_hooks"] = mod
        import antenv
        antenv.axon_hooks = mod
        mod.set_axon_ntff_profile_hook(
            _ntff_profile_via_ctypes("/opt/axon/libaxon_pjrt.so"))
        return True
    except Exception:
        return False


def kernel(**inputs):
    from concourse.bass_utils import run_bass_kernel_spmd

    nc = _get_nc()
    in_maps = _make_in_maps(inputs)
    trace = bool(int(os.environ.get("KERNEL_TRACE", "0")))
    if trace and not _ensure_ntff_hook():
        trace = False
    res = run_bass_kernel_spmd(nc, in_maps, list(range(N_CORES)), trace=trace)
    if trace:
        _CACHE["last_result"] = res
    out = np.concatenate([res.results[i]["out"] for i in range(N_CORES)], axis=0)
    return out.reshape(np.asarray(inputs["x"]).shape).astype(np.float32)


# revision 7
# speedup vs baseline: 1.2896x; 1.0128x over previous
"""Trainium2 Bass kernel: ConvNeXt MLP + parallel MoE-LoRA (data-parallel over tokens).

Math per token t (D=512, Dh=2048, E=3 experts, r=8, top-k=2):
    base = gelu(x @ W1 + b1) @ W2 + b2
    g_e  = gelu(x @ w_down[e]) * wts[e, t]          (wts from top-k routing)
    out  = base + sum_e g_e @ w_up[e]

Strategy (per NeuronCore, 8 cores data-parallel on the token dim):
  - tokens tiled 128 at a time; supergroups of 4 tiles (512 tokens) so the
    MM1 moving free dim is 512.
  - x tile [128t, 512d] is cast to bf16 and PE-transposed into xT [128d, t].
  - MM1: hT[h,t] = W1_chunk.T @ xT   (feature-major hidden), fused
    bias+gelu on ScalarE into actT (bf16).
  - MM2: out[t,d] accumulates 16 h-chunks (lhsT = actT slices) + the
    MoE-LoRA rank-24 matmul in one PSUM accumulation group; b2 is added
    during the PSUM->SBUF drain from a replicated [128,512] bias tile.
  - LoRA: g[t,24] = gelu(xT.T @ wdown_all), scaled per-expert by routing
    weights (per-partition scalars), PE-transposed, matmul'd with
    wup_all[24,512] into the same PSUM accumulator.
  - routing weights wts[e,t] = sum_k probs[t,k]*(idx[t,k]==e) computed on
    device in a small DVE prologue over all tokens at once.
  - matmuls run in bf16 (full PE rate, fast weight load); accumulation is
    always fp32 in PSUM.
"""

import os
import numpy as np

P = 128
D = 512
DH = 2048
E = 3
R = 8
ER = E * R  # 24
NH = DH // P  # 16
NDC = D // P  # 4
N_CORES = 8
T_FULL = 64 * 28 * 28  # 50176
TC = T_FULL // N_CORES  # 6272
GROUP_TILES = 4

_CACHE = {}


def _build(tc_tokens, use_gelu=True):
    import concourse.bacc as bacc
    import concourse.tile as tile
    import concourse.mybir as mybir
    from contextlib import ExitStack

    f32 = mybir.dt.float32
    bf16 = mybir.dt.bfloat16
    i32 = mybir.dt.int32
    AF = mybir.ActivationFunctionType
    act_fn = AF.Gelu if use_gelu else AF.Relu
    OP = mybir.AluOpType

    nt = tc_tokens // P  # token tiles
    assert tc_tokens % P == 0

    nc = bacc.Bacc("TRN2", target_bir_lowering=False, debug=False,
                   num_devices=N_CORES)

    x = nc.dram_tensor("x", [tc_tokens, D], f32, kind="ExternalInput").ap()
    w1 = nc.dram_tensor("w1", [D, DH], f32, kind="ExternalInput").ap()
    w2 = nc.dram_tensor("w2", [DH, D], f32, kind="ExternalInput").ap()
    b1 = nc.dram_tensor("b1", [DH], f32, kind="ExternalInput").ap()
    b2 = nc.dram_tensor("b2", [D], f32, kind="ExternalInput").ap()
    wd = nc.dram_tensor("wd", [D, ER], f32, kind="ExternalInput").ap()
    wu = nc.dram_tensor("wu", [ER, D], f32, kind="ExternalInput").ap()
    tkp = nc.dram_tensor("tkp", [tc_tokens, 2], f32, kind="ExternalInput").ap()
    tki = nc.dram_tensor("tki", [tc_tokens, 4], i32, kind="ExternalInput").ap()
    ident_d = nc.dram_tensor("ident", [P, P], f32, kind="ExternalInput").ap()
    out = nc.dram_tensor("out", [tc_tokens, D], f32, kind="ExternalOutput").ap()

    with tile.TileContext(nc) as tc, ExitStack() as ctx:
        cons = ctx.enter_context(tc.tile_pool(name="cons", bufs=1))
        stg = ctx.enter_context(tc.tile_pool(name="stg", bufs=4))
        xin = ctx.enter_context(tc.tile_pool(name="xin", bufs=12))
        xbp = ctx.enter_context(tc.tile_pool(name="xbp", bufs=4))
        xtp = ctx.enter_context(tc.tile_pool(name="xtp", bufs=2))
        actp = ctx.enter_context(tc.tile_pool(name="actp", bufs=2))
        outp = ctx.enter_context(tc.tile_pool(name="outp", bufs=4))
        gp = ctx.enter_context(tc.tile_pool(name="gp", bufs=2))
        ps_xt = ctx.enter_context(tc.tile_pool(name="ps_xt", bufs=2, space="PSUM"))
        ps_h = ctx.enter_context(tc.tile_pool(name="ps_h", bufs=2, space="PSUM"))
        ps_o = ctx.enter_context(tc.tile_pool(name="ps_o", bufs=2, space="PSUM"))
        ps_g = ctx.enter_context(tc.tile_pool(name="ps_g", bufs=1, space="PSUM"))
        ps_gt = ctx.enter_context(tc.tile_pool(name="ps_gt", bufs=1, space="PSUM"))

        # ---- identity (needed by the very first transposes) ----
        ident_sb = cons.tile([P, P], f32)
        nc.sync.dma_start(ident_sb[:], ident_d)
        ident_b = cons.tile([P, P], bf16)
        nc.vector.tensor_copy(ident_b[:], ident_sb[:])

        # ---- prefetch x tiles of group 0 before the weight DMAs ----
        prefetched = {}
        for i in range(min(2 * GROUP_TILES, nt)):
            x_t = xin.tile([P, D], f32, tag="x_t", name=f"x_t{i}")
            nc.sync.dma_start(x_t[:], x[i * P:(i + 1) * P, :])
            prefetched[i] = x_t

        # ---- weights: staged fp32 DMA -> bf16 cast ----
        W1s = cons.tile([P, NDC * DH], bf16)
        for c in range(NDC):
            s = stg.tile([P, DH], f32, tag="stage", name=f"stg_w1_{c}")
            nc.sync.dma_start(s[:], w1[c * P:(c + 1) * P, :])
            nc.vector.tensor_copy(W1s[:, c * DH:(c + 1) * DH], s[:])
        W2s = cons.tile([P, NH * D], bf16)
        for h in range(NH):
            s = stg.tile([P, D], f32, tag="stage", name=f"stg_w2_{h}")
            nc.sync.dma_start(s[:], w2[h * P:(h + 1) * P, :])
            nc.vector.tensor_copy(W2s[:, h * D:(h + 1) * D], s[:])
        wdr = cons.tile([P, NDC * ER], bf16)
        for c in range(NDC):
            s = stg.tile([P, ER], f32, tag="stage", name=f"stg_wd_{c}")
            nc.sync.dma_start(s[:], wd[c * P:(c + 1) * P, :])
            nc.vector.tensor_copy(wdr[:, c * ER:(c + 1) * ER], s[:])
        wur = cons.tile([ER, D], bf16)
        s = stg.tile([ER, D], f32, tag="stage", name="stg_wu")
        nc.sync.dma_start(s[:], wu)
        nc.vector.tensor_copy(wur[:], s[:])
        b1_sb = cons.tile([P, NH], f32)
        nc.sync.dma_start(b1_sb[:], b1.rearrange("(c p) -> p c", p=P))

        # ---- replicated b2 [128, 512] via rank-1 ones x b2 matmul ----
        ones_b = cons.tile([1, P], bf16)
        nc.vector.memset(ones_b[:], 1.0)
        b2s = stg.tile([1, D], f32, tag="stage", name="stg_b2")
        nc.sync.dma_start(b2s[:], b2.rearrange("(o d) -> o d", o=1))
        b2b = cons.tile([1, D], bf16)
        nc.vector.tensor_copy(b2b[:], b2s[:])
        ps_b2 = ps_o.tile([P, D], f32, tag="ps_out", name="ps_b2rep")
        nc.tensor.matmul(ps_b2[:], ones_b[:], b2b[:], start=True, stop=True)
        b2rep = cons.tile([P, D], f32)
        nc.vector.tensor_copy(b2rep[:], ps_b2[:])

        # ---- routing weights prologue: wts[e, tile, p] ----
        tp_sb = cons.tile([P, nt * 2], f32)
        nc.sync.dma_start(tp_sb.rearrange("p (n k) -> p n k", k=2),
                          tkp.rearrange("(n p) k -> p n k", p=P))
        ti_sb = cons.tile([P, nt * 4], i32)
        nc.sync.dma_start(ti_sb.rearrange("p (n k) -> p n k", k=4),
                          tki.rearrange("(n p) k -> p n k", p=P))
        idxf = cons.tile([P, nt * 2], f32)
        iv = ti_sb.rearrange("p (n k two) -> p n k two", k=2, two=2)
        nc.vector.tensor_copy(
            idxf.rearrange("p (n k one) -> p n k one", k=2, one=1),
            iv[:, :, :, 0:1])
        wts = cons.tile([P, E * nt], f32)
        for e in range(E):
            eq = cons.tile([P, nt * 2], f32, tag="eq", name=f"eq{e}")
            nc.vector.tensor_scalar(eq[:], idxf[:], float(e), None,
                                    op0=OP.is_equal)
            nc.vector.tensor_tensor(eq[:], eq[:], tp_sb[:], op=OP.mult)
            nc.vector.tensor_reduce(wts[:, e * nt:(e + 1) * nt],
                                    eq.rearrange("p (n k) -> p n k", k=2),
                                    axis=mybir.AxisListType.X, op=OP.add)

        # ---- main loop over supergroups ----
        t0 = 0
        while t0 < nt:
            ng = min(GROUP_TILES, nt - t0)
            G = ng * P

            x_ts = []
            for i in range(ng):
                tt = t0 + i
                if tt in prefetched:
                    x_t = prefetched.pop(tt)
                else:
                    x_t = xin.tile([P, D], f32, tag="x_t", name=f"x_t{tt}")
                    nc.sync.dma_start(x_t[:], x[tt * P:(tt + 1) * P, :])
                x_ts.append(x_t)
            # prefetch the group after next so DMA stays ahead of compute
            for pf in range(t0 + 2 * GROUP_TILES, min(t0 + 3 * GROUP_TILES, nt)):
                if pf not in prefetched:
                    x_t = xin.tile([P, D], f32, tag="x_t", name=f"x_t{pf}")
                    nc.sync.dma_start(x_t[:], x[pf * P:(pf + 1) * P, :])
                    prefetched[pf] = x_t

            # cast x to bf16, transpose -> xT [128d, (c, t)] bf16
            xT = xtp.tile([P, NDC * G], bf16, tag="xT", name=f"xT{t0}")
            for i in range(ng):
                xb = xbp.tile([P, D], bf16, tag="xb", name=f"xb{t0 + i}")
                nc.vector.tensor_copy(xb[:], x_ts[i][:])
                px = ps_xt.tile([P, 512], bf16, tag="ps_x", name=f"ps_x{t0 + i}")
                for c in range(NDC):
                    nc.tensor.matmul(px[:, c * P:(c + 1) * P],
                                     xb[:, c * P:(c + 1) * P],
                                     ident_b[:],
                                     is_transpose=True,
                                     start=(c == 0), stop=(c == NDC - 1))
                nc.vector.tensor_copy(
                    xT.rearrange("p (c g) -> p c g", c=NDC)[:, :, i * P:(i + 1) * P],
                    px.rearrange("p (c g) -> p c g", c=NDC))

            # MM1 + bias + gelu -> actT [128h, (h_chunk, t)] bf16
            actT = actp.tile([P, NH * G], bf16, tag="actT", name=f"actT{t0}")
            for h in range(NH):
                ph = ps_h.tile([P, 512], f32, tag="ps_hT", name=f"ps_hT{t0}_{h}")
                for c in range(NDC):
                    nc.tensor.matmul(
                        ph[:, :G],
                        W1s[:, c * DH + h * P: c * DH + (h + 1) * P],
                        xT[:, c * G:(c + 1) * G],
                        start=(c == 0), stop=(c == NDC - 1))
                nc.scalar.activation(actT[:, h * G:(h + 1) * G], ph[:, :G],
                                     act_fn, bias=b1_sb[:, h:h + 1], scale=1.0)

            # MM2 + LoRA per token tile
            for i in range(ng):
                tt = t0 + i
                po = ps_o.tile([P, D], f32, tag="ps_out", name=f"ps_out{tt}")
                for h in range(NH):
                    nc.tensor.matmul(
                        po[:],
                        actT[:, h * G + i * P: h * G + (i + 1) * P],
                        W2s[:, h * D:(h + 1) * D],
                        start=(h == 0), stop=False)
                # LoRA down: g[t, 24]
                pg = ps_g.tile([P, ER], f32, tag="ps_lg", name=f"ps_lg{tt}")
                for c in range(NDC):
                    nc.tensor.matmul(
                        pg[:],
                        xT[:, c * G + i * P: c * G + (i + 1) * P],
                        wdr[:, c * ER:(c + 1) * ER],
                        start=(c == 0), stop=(c == NDC - 1))
                g_sb = gp.tile([P, ER], f32, tag="g_sb", name=f"g_sb{tt}")
                nc.scalar.activation(g_sb[:], pg[:], act_fn)
                g2 = gp.tile([P, ER], bf16, tag="g2", name=f"g2{tt}")
                for e in range(E):
                    nc.vector.tensor_scalar(
                        g2[:, e * R:(e + 1) * R], g_sb[:, e * R:(e + 1) * R],
                        wts[:, e * nt + tt: e * nt + tt + 1], None, op0=OP.mult)
                pgt = ps_gt.tile([ER, P], bf16, tag="ps_lgt", name=f"ps_lgt{tt}")
                nc.tensor.matmul(pgt[:], g2[:], ident_b[:], is_transpose=True)
                gt_sb = gp.tile([ER, P], bf16, tag="gt_sb", name=f"gt_sb{tt}")
                nc.vector.tensor_copy(gt_sb[:], pgt[:])
                # LoRA up into the same accumulator (closes the group)
                nc.tensor.matmul(po[:], gt_sb[:], wur[:],
                                 start=False, stop=True)

                o_sb = outp.tile([P, D], f32, tag="o_sb", name=f"o_sb{tt}")
                nc.vector.tensor_tensor(o_sb[:], po[:], b2rep[:], op=OP.add)
                nc.sync.dma_start(out[tt * P:(tt + 1) * P, :], o_sb[:])

            t0 += ng

    nc.compile()
    return nc


def _get_nc():
    key = ("full", TC)
    if key not in _CACHE:
        _CACHE[key] = _build(TC, use_gelu=True)
    return _CACHE[key]


def _make_in_maps(inputs, tc_tokens=TC, n_cores=N_CORES):
    x = np.ascontiguousarray(inputs["x"], dtype=np.float32)
    T = x.size // D
    x_flat = x.reshape(T, D)
    W1 = np.ascontiguousarray(inputs["W1"], dtype=np.float32)
    W2 = np.ascontiguousarray(inputs["W2"], dtype=np.float32)
    b1 = np.ascontiguousarray(inputs["b1"], dtype=np.float32)
    b2 = np.ascontiguousarray(inputs["b2"], dtype=np.float32)
    wdn = np.ascontiguousarray(
        np.asarray(inputs["w_down"], dtype=np.float32).transpose(1, 0, 2).reshape(D, ER))
    wup = np.ascontiguousarray(
        np.asarray(inputs["w_up"], dtype=np.float32).reshape(ER, D))
    tkp = np.ascontiguousarray(inputs["topk_probs"], dtype=np.float32)
    tki_in = np.asarray(inputs["topk_indices"])
    tki = np.zeros((T, 4), dtype=np.int32)
    tki[:, 0] = tki_in[:, 0]
    tki[:, 2] = tki_in[:, 1]
    ident = np.eye(P, dtype=np.float32)

    in_maps = []
    for c in range(n_cores):
        sl = slice(c * tc_tokens, (c + 1) * tc_tokens)
        in_maps.append(dict(
            x=np.ascontiguousarray(x_flat[sl]), w1=W1, w2=W2, b1=b1, b2=b2,
            wd=wdn, wu=wup, tkp=np.ascontiguousarray(tkp[sl]),
            tki=np.ascontiguousarray(tki[sl]), ident=ident))
    return in_maps


def _ensure_ntff_hook():
    """Register the axon NTFF profile hook if the image's antenv lacks it."""
    import sys
    import types
    try:
        from antenv.axon_hooks import get_axon_ntff_profile_hook  # noqa: F401
        return True
    except ImportError:
        pass
    try:
        from trn_agent_boot.trn_boot import _ntff_profile_via_ctypes
        mod = types.ModuleType("antenv.axon_hooks")
        _hook = [None]
        mod.set_axon_ntff_profile_hook = lambda h: _hook.__setitem__(0, h)
        mod.get_axon_ntff_profile_hook = lambda: _hook[0]
        sys.modules["antenv.axon_hooks"] = mod
        import antenv
        antenv.axon_hooks = mod
        mod.set_axon_ntff_profile_hook(
            _ntff_profile_via_ctypes("/opt/axon/libaxon_pjrt.so"))
        return True
    except Exception:
        return False


def kernel(**inputs):
    from concourse.bass_utils import run_bass_kernel_spmd

    nc = _get_nc()
    in_maps = _make_in_maps(inputs)
    trace = bool(int(os.environ.get("KERNEL_TRACE", "0")))
    if trace and not _ensure_ntff_hook():
        trace = False
    res = run_bass_kernel_spmd(nc, in_maps, list(range(N_CORES)), trace=trace)
    if trace:
        _CACHE["last_result"] = res
    out = np.concatenate([res.results[i]["out"] for i in range(N_CORES)], axis=0)
    return out.reshape(np.asarray(inputs["x"]).shape).astype(np.float32)


# revision 8
# speedup vs baseline: 1.2956x; 1.0046x over previous
"""Trainium2 Bass kernel: ConvNeXt MLP + parallel MoE-LoRA (data-parallel over tokens).

Math per token t (D=512, Dh=2048, E=3 experts, r=8, top-k=2):
    base = gelu(x @ W1 + b1) @ W2 + b2
    g_e  = gelu(x @ w_down[e]) * wts[e, t]          (wts from top-k routing)
    out  = base + sum_e g_e @ w_up[e]

Strategy (per NeuronCore, 8 cores data-parallel on the token dim):
  - tokens tiled 128 at a time; supergroups of 4 tiles (512 tokens) so the
    MM1 moving free dim is 512.
  - x tile [128t, 512d] is cast to bf16 and PE-transposed into xT [128d, t].
  - MM1: hT[h,t] = W1_chunk.T @ xT   (feature-major hidden), fused
    bias+gelu on ScalarE into actT (bf16).
  - MM2: out[t,d] accumulates 16 h-chunks (lhsT = actT slices) + the
    MoE-LoRA rank-24 matmul in one PSUM accumulation group; b2 is added
    during the PSUM->SBUF drain from a replicated [128,512] bias tile.
  - LoRA: g[t,24] = gelu(xT.T @ wdown_all), scaled per-expert by routing
    weights (per-partition scalars), PE-transposed, matmul'd with
    wup_all[24,512] into the same PSUM accumulator.
  - routing weights wts[e,t] = sum_k probs[t,k]*(idx[t,k]==e) computed on
    device in a small DVE prologue over all tokens at once.
  - matmuls run in bf16 (full PE rate, fast weight load); accumulation is
    always fp32 in PSUM.
"""

import os
import numpy as np

P = 128
D = 512
DH = 2048
E = 3
R = 8
ER = E * R  # 24
NH = DH // P  # 16
NDC = D // P  # 4
N_CORES = 8
T_FULL = 64 * 28 * 28  # 50176
TC = T_FULL // N_CORES  # 6272
GROUP_TILES = 4

_CACHE = {}


def _build(tc_tokens, use_gelu=True):
    import concourse.bacc as bacc
    import concourse.tile as tile
    import concourse.mybir as mybir
    from contextlib import ExitStack

    f32 = mybir.dt.float32
    bf16 = mybir.dt.bfloat16
    i32 = mybir.dt.int32
    AF = mybir.ActivationFunctionType
    act_fn = AF.Gelu if use_gelu else AF.Relu
    OP = mybir.AluOpType

    nt = tc_tokens // P  # token tiles
    assert tc_tokens % P == 0

    nc = bacc.Bacc("TRN2", target_bir_lowering=False, debug=False,
                   num_devices=N_CORES)

    x = nc.dram_tensor("x", [tc_tokens, D], f32, kind="ExternalInput").ap()
    w1 = nc.dram_tensor("w1", [D, DH], f32, kind="ExternalInput").ap()
    w2 = nc.dram_tensor("w2", [DH, D], f32, kind="ExternalInput").ap()
    b1 = nc.dram_tensor("b1", [DH], f32, kind="ExternalInput").ap()
    b2 = nc.dram_tensor("b2", [D], f32, kind="ExternalInput").ap()
    wd = nc.dram_tensor("wd", [D, ER], f32, kind="ExternalInput").ap()
    wu = nc.dram_tensor("wu", [ER, D], f32, kind="ExternalInput").ap()
    tkp = nc.dram_tensor("tkp", [tc_tokens, 2], f32, kind="ExternalInput").ap()
    tki = nc.dram_tensor("tki", [tc_tokens, 4], i32, kind="ExternalInput").ap()
    ident_d = nc.dram_tensor("ident", [P, P], f32, kind="ExternalInput").ap()
    out = nc.dram_tensor("out", [tc_tokens, D], f32, kind="ExternalOutput").ap()

    with tile.TileContext(nc) as tc, ExitStack() as ctx:
        cons = ctx.enter_context(tc.tile_pool(name="cons", bufs=1))
        stg = ctx.enter_context(tc.tile_pool(name="stg", bufs=4))
        xin = ctx.enter_context(tc.tile_pool(name="xin", bufs=12))
        xbp = ctx.enter_context(tc.tile_pool(name="xbp", bufs=4))
        xtp = ctx.enter_context(tc.tile_pool(name="xtp", bufs=2))
        actp = ctx.enter_context(tc.tile_pool(name="actp", bufs=2))
        outp = ctx.enter_context(tc.tile_pool(name="outp", bufs=4))
        gp = ctx.enter_context(tc.tile_pool(name="gp", bufs=8))
        ps_xt = ctx.enter_context(tc.tile_pool(name="ps_xt", bufs=2, space="PSUM"))
        ps_h = ctx.enter_context(tc.tile_pool(name="ps_h", bufs=2, space="PSUM"))
        ps_o = ctx.enter_context(tc.tile_pool(name="ps_o", bufs=2, space="PSUM"))
        ps_g = ctx.enter_context(tc.tile_pool(name="ps_g", bufs=1, space="PSUM"))
        ps_gt = ctx.enter_context(tc.tile_pool(name="ps_gt", bufs=1, space="PSUM"))

        # ---- identity (needed by the very first transposes) ----
        ident_sb = cons.tile([P, P], f32)
        nc.sync.dma_start(ident_sb[:], ident_d)
        ident_b = cons.tile([P, P], bf16)
        nc.vector.tensor_copy(ident_b[:], ident_sb[:])

        # ---- prefetch x tiles of group 0 before the weight DMAs ----
        prefetched = {}
        for i in range(min(2 * GROUP_TILES, nt)):
            x_t = xin.tile([P, D], f32, tag="x_t", name=f"x_t{i}")
            nc.sync.dma_start(x_t[:], x[i * P:(i + 1) * P, :])
            prefetched[i] = x_t

        # ---- weights: staged fp32 DMA -> bf16 cast ----
        W1s = cons.tile([P, NDC * DH], bf16)
        for c in range(NDC):
            s = stg.tile([P, DH], f32, tag="stage", name=f"stg_w1_{c}")
            nc.sync.dma_start(s[:], w1[c * P:(c + 1) * P, :])
            nc.vector.tensor_copy(W1s[:, c * DH:(c + 1) * DH], s[:])
        W2s = cons.tile([P, NH * D], bf16)
        for h in range(NH):
            s = stg.tile([P, D], f32, tag="stage", name=f"stg_w2_{h}")
            nc.sync.dma_start(s[:], w2[h * P:(h + 1) * P, :])
            nc.scalar.activation(W2s[:, h * D:(h + 1) * D], s[:], AF.Copy)
        wdr = cons.tile([P, NDC * ER], bf16)
        for c in range(NDC):
            s = stg.tile([P, ER], f32, tag="stage", name=f"stg_wd_{c}")
            nc.sync.dma_start(s[:], wd[c * P:(c + 1) * P, :])
            nc.scalar.activation(wdr[:, c * ER:(c + 1) * ER], s[:], AF.Copy)
        wur = cons.tile([ER, D], bf16)
        s = stg.tile([ER, D], f32, tag="stage", name="stg_wu")
        nc.sync.dma_start(s[:], wu)
        nc.scalar.activation(wur[:], s[:], AF.Copy)
        b1_sb = cons.tile([P, NH], f32)
        nc.sync.dma_start(b1_sb[:], b1.rearrange("(c p) -> p c", p=P))

        # ---- replicated b2 [128, 512] via rank-1 ones x b2 matmul ----
        ones_b = cons.tile([1, P], bf16)
        nc.vector.memset(ones_b[:], 1.0)
        b2s = stg.tile([1, D], f32, tag="stage", name="stg_b2")
        nc.sync.dma_start(b2s[:], b2.rearrange("(o d) -> o d", o=1))
        b2b = cons.tile([1, D], bf16)
        nc.vector.tensor_copy(b2b[:], b2s[:])
        ps_b2 = ps_o.tile([P, D], f32, tag="ps_out", name="ps_b2rep")
        nc.tensor.matmul(ps_b2[:], ones_b[:], b2b[:], start=True, stop=True)
        b2rep = cons.tile([P, D], f32)
        nc.vector.tensor_copy(b2rep[:], ps_b2[:])

        # ---- routing weights prologue: wts[e, tile, p] ----
        tp_sb = cons.tile([P, nt * 2], f32)
        nc.sync.dma_start(tp_sb.rearrange("p (n k) -> p n k", k=2),
                          tkp.rearrange("(n p) k -> p n k", p=P))
        ti_sb = cons.tile([P, nt * 4], i32)
        nc.sync.dma_start(ti_sb.rearrange("p (n k) -> p n k", k=4),
                          tki.rearrange("(n p) k -> p n k", p=P))
        idxf = cons.tile([P, nt * 2], f32)
        iv = ti_sb.rearrange("p (n k two) -> p n k two", k=2, two=2)
        nc.vector.tensor_copy(
            idxf.rearrange("p (n k one) -> p n k one", k=2, one=1),
            iv[:, :, :, 0:1])
        wts = cons.tile([P, E * nt], f32)
        for e in range(E):
            eq = cons.tile([P, nt * 2], f32, tag="eq", name=f"eq{e}")
            nc.vector.tensor_scalar(eq[:], idxf[:], float(e), None,
                                    op0=OP.is_equal)
            nc.vector.tensor_tensor(eq[:], eq[:], tp_sb[:], op=OP.mult)
            nc.vector.tensor_reduce(wts[:, e * nt:(e + 1) * nt],
                                    eq.rearrange("p (n k) -> p n k", k=2),
                                    axis=mybir.AxisListType.X, op=OP.add)

        # ---- main loop over supergroups ----
        t0 = 0
        while t0 < nt:
            ng = min(GROUP_TILES, nt - t0)
            G = ng * P

            x_ts = []
            for i in range(ng):
                tt = t0 + i
                if tt in prefetched:
                    x_t = prefetched.pop(tt)
                else:
                    x_t = xin.tile([P, D], f32, tag="x_t", name=f"x_t{tt}")
                    nc.sync.dma_start(x_t[:], x[tt * P:(tt + 1) * P, :])
                x_ts.append(x_t)
            # prefetch the group after next so DMA stays ahead of compute
            for pf in range(t0 + 2 * GROUP_TILES, min(t0 + 3 * GROUP_TILES, nt)):
                if pf not in prefetched:
                    x_t = xin.tile([P, D], f32, tag="x_t", name=f"x_t{pf}")
                    nc.sync.dma_start(x_t[:], x[pf * P:(pf + 1) * P, :])
                    prefetched[pf] = x_t

            # cast x to bf16, transpose -> xT [128d, (c, t)] bf16
            xT = xtp.tile([P, NDC * G], bf16, tag="xT", name=f"xT{t0}")
            for i in range(ng):
                xb = xbp.tile([P, D], bf16, tag="xb", name=f"xb{t0 + i}")
                nc.vector.tensor_copy(xb[:], x_ts[i][:])
                px = ps_xt.tile([P, 512], bf16, tag="ps_x", name=f"ps_x{t0 + i}")
                for c in range(NDC):
                    nc.tensor.matmul(px[:, c * P:(c + 1) * P],
                                     xb[:, c * P:(c + 1) * P],
                                     ident_b[:],
                                     is_transpose=True,
                                     start=(c == 0), stop=(c == NDC - 1))
                nc.vector.tensor_copy(
                    xT.rearrange("p (c g) -> p c g", c=NDC)[:, :, i * P:(i + 1) * P],
                    px.rearrange("p (c g) -> p c g", c=NDC))

            # MM1 + bias + gelu -> actT [128h, (h_chunk, t)] bf16
            actT = actp.tile([P, NH * G], bf16, tag="actT", name=f"actT{t0}")
            for h in range(NH):
                ph = ps_h.tile([P, 512], f32, tag="ps_hT", name=f"ps_hT{t0}_{h}")
                for c in range(NDC):
                    nc.tensor.matmul(
                        ph[:, :G],
                        W1s[:, c * DH + h * P: c * DH + (h + 1) * P],
                        xT[:, c * G:(c + 1) * G],
                        start=(c == 0), stop=(c == NDC - 1))
                nc.scalar.activation(actT[:, h * G:(h + 1) * G], ph[:, :G],
                                     act_fn, bias=b1_sb[:, h:h + 1], scale=1.0)

            # LoRA chain for all tiles of the group (hides under MM1/MM2)
            gts = []
            for i in range(ng):
                tt = t0 + i
                pg = ps_g.tile([P, ER], f32, tag="ps_lg", name=f"ps_lg{tt}")
                for c in range(NDC):
                    nc.tensor.matmul(
                        pg[:],
                        xT[:, c * G + i * P: c * G + (i + 1) * P],
                        wdr[:, c * ER:(c + 1) * ER],
                        start=(c == 0), stop=(c == NDC - 1))
                g_sb = gp.tile([P, ER], f32, tag="g_sb", name=f"g_sb{tt}")
                nc.scalar.activation(g_sb[:], pg[:], act_fn)
                g2 = gp.tile([P, ER], bf16, tag="g2", name=f"g2{tt}")
                for e in range(E):
                    nc.vector.tensor_scalar(
                        g2[:, e * R:(e + 1) * R], g_sb[:, e * R:(e + 1) * R],
                        wts[:, e * nt + tt: e * nt + tt + 1], None, op0=OP.mult)
                pgt = ps_gt.tile([ER, P], bf16, tag="ps_lgt", name=f"ps_lgt{tt}")
                nc.tensor.matmul(pgt[:], g2[:], ident_b[:], is_transpose=True)
                gt_sb = gp.tile([ER, P], bf16, tag="gt_sb", name=f"gt_sb{tt}")
                nc.vector.tensor_copy(gt_sb[:], pgt[:])
                gts.append(gt_sb)

            # MM2 per token tile
            for i in range(ng):
                tt = t0 + i
                po = ps_o.tile([P, D], f32, tag="ps_out", name=f"ps_out{tt}")
                for h in range(NH):
                    nc.tensor.matmul(
                        po[:],
                        actT[:, h * G + i * P: h * G + (i + 1) * P],
                        W2s[:, h * D:(h + 1) * D],
                        start=(h == 0), stop=False)
                # LoRA up into the same accumulator (closes the group)
                nc.tensor.matmul(po[:], gts[i][:], wur[:],
                                 start=False, stop=True)

                o_sb = outp.tile([P, D], f32, tag="o_sb", name=f"o_sb{tt}")
                nc.vector.tensor_tensor(o_sb[:], po[:], b2rep[:], op=OP.add)
                nc.sync.dma_start(out[tt * P:(tt + 1) * P, :], o_sb[:])

            t0 += ng

    nc.compile()
    return nc


def _get_nc():
    key = ("full", TC)
    if key not in _CACHE:
        _CACHE[key] = _build(TC, use_gelu=True)
    return _CACHE[key]


def _make_in_maps(inputs, tc_tokens=TC, n_cores=N_CORES):
    x = np.ascontiguousarray(inputs["x"], dtype=np.float32)
    T = x.size // D
    x_flat = x.reshape(T, D)
    W1 = np.ascontiguousarray(inputs["W1"], dtype=np.float32)
    W2 = np.ascontiguousarray(inputs["W2"], dtype=np.float32)
    b1 = np.ascontiguousarray(inputs["b1"], dtype=np.float32)
    b2 = np.ascontiguousarray(inputs["b2"], dtype=np.float32)
    wdn = np.ascontiguousarray(
        np.asarray(inputs["w_down"], dtype=np.float32).transpose(1, 0, 2).reshape(D, ER))
    wup = np.ascontiguousarray(
        np.asarray(inputs["w_up"], dtype=np.float32).reshape(ER, D))
    tkp = np.ascontiguousarray(inputs["topk_probs"], dtype=np.float32)
    tki_in = np.asarray(inputs["topk_indices"])
    tki = np.zeros((T, 4), dtype=np.int32)
    tki[:, 0] = tki_in[:, 0]
    tki[:, 2] = tki_in[:, 1]
    ident = np.eye(P, dtype=np.float32)

    in_maps = []
    for c in range(n_cores):
        sl = slice(c * tc_tokens, (c + 1) * tc_tokens)
        in_maps.append(dict(
            x=np.ascontiguousarray(x_flat[sl]), w1=W1, w2=W2, b1=b1, b2=b2,
            wd=wdn, wu=wup, tkp=np.ascontiguousarray(tkp[sl]),
            tki=np.ascontiguousarray(tki[sl]), ident=ident))
    return in_maps


def _ensure_ntff_hook():
    """Register the axon NTFF profile hook if the image's antenv lacks it."""
    import sys
    import types
    try:
        from antenv.axon_hooks import get_axon_ntff_profile_hook  # noqa: F401
        return True
    except ImportError:
        pass
    try:
        from trn_agent_boot.trn_boot import _ntff_profile_via_ctypes
        mod = types.ModuleType("antenv.axon_hooks")
        _hook = [None]
        mod.set_axon_ntff_profile_hook = lambda h: _hook.__setitem__(0, h)
        mod.get_axon_ntff_profile_hook = lambda: _hook[0]
        sys.modules["antenv.axon_hooks"] = mod
        import antenv
        antenv.axon_hooks = mod
        mod.set_axon_ntff_profile_hook(
            _ntff_profile_via_ctypes("/opt/axon/libaxon_pjrt.so"))
        return True
    except Exception:
        return False


def kernel(**inputs):
    from concourse.bass_utils import run_bass_kernel_spmd

    nc = _get_nc()
    in_maps = _make_in_maps(inputs)
    trace = bool(int(os.environ.get("KERNEL_TRACE", "0")))
    if trace and not _ensure_ntff_hook():
        trace = False
    res = run_bass_kernel_spmd(nc, in_maps, list(range(N_CORES)), trace=trace)
    if trace:
        _CACHE["last_result"] = res
    out = np.concatenate([res.results[i]["out"] for i in range(N_CORES)], axis=0)
    return out.reshape(np.asarray(inputs["x"]).shape).astype(np.float32)
